# revision 2
# baseline (speedup 1.0000x reference)
"""Batched KDE kernel for Trainium2 (8 NeuronCores, SPMD) — v3.

Problem: out[b, n] = sum_m exp(-||Xq[b,n] - Xf[b,m]||^2 / bw[b])
  with Silverman bandwidth bw[b] from Xf; b=4, n=m=4096, d=32.

Sharding: data-parallel over batch b (4 batches x 2 shards of query rows
= 8 cores). Each core: n_shard=2048 query rows x full m=4096 fit set.

Design (vs v1's two-bf16-matmul + ACT-only exp):
  * ONE bf16 matmul per 512-col psum chunk: K=99 rows packing the 2-way
    bf16 split cross terms q1*F1 + q1*F2 + q2*F1 (96 rows) plus 3 rows
    (-1 x nm_i) subtracting a 3-way bf16 split of nmu2. The F side
    carries the full scale 2*2^23*log2(e)/bw, so psum = y*2^23 where
    y = (2dot - nmu2)*log2e/bw  (log2 of the exp result, un-biased).
  * The exp elementwise wall (the kernel's roofline) is split across
    BOTH the Scalar (ACT) and Vector (DVE) engines. Groups of
    [128, 2048] psum are assigned per a host pattern:
      'A' : ACT exp over all 2048 cols (accum_out = m-reduction)
      'D' : DVE custom-op exp over all 2048 cols
      'M' : ACT 1536 (banks 0-2) + DVE 512 (bank 3), concurrent
    ACT: exp(psum*scale + bias_n), scale = ln2/2^23, bias_n = -nx2/bw.
    DVE: pass1 = relu(psum + C0) -> int32 (C0 = (-nx2*log2e/bw + 127 +
    log2|a3|)*2^23); pass2 bitcasts the int back to float e2 =
    2^(E-127)*(1+f), rebuilds t = 1+f by OR-ing with 1.0f (E < 128 so
    the exponent field becomes exactly 127), and applies the cubic
    mantissa correction 2^f/(1+f) ~ |a3|*(((A2-t)t+A1)t+A0) in Horner
    form (|a3| folded into C0), with a fused accum over m. pass2 runs
    one group behind pass1 so its RAW semaphore wait is pre-satisfied.
  * Input DMAs are issued in parallel from GPSIMD and SYNC; xqn is
    split in 4 so the on-device nx2/bias chain starts early, sliced in
    4 so group 0's bias is ready well before the first psum group.
Host does sharding/layout/packing plus the 4 scalar bandwidth values
(the global quantile needs a sort, pathological on-device); nx2 is
computed on-device from the raw query rows.
"""

import numpy as np

B, N, M, D = 4, 4096, 4096, 32
NCORES = 8
SHARDS_PER_BATCH = NCORES // B  # 2
NSHARD = N // SHARDS_PER_BATCH  # 2048
NT = NSHARD // 128  # 16 n-tiles per core
MCHUNK = 512  # one psum bank
GW = 2048  # group width (4 banks)
NG = NT * (M // GW)  # 32 groups per core
K = 99  # matmul contraction rows

# cubic mantissa-correction, Horner form: for t = 1+f in [1,2),
#   2^f/(1+f) ~ |a3| * (((A2 - t)*t + A1)*t + A0),  |a3| folded into bias
A2_C = 6.688728929456908
A1_C = -13.073163690873669
A0_C = 17.009672799660986
LOG2_NEG_C3 = -3.267369783285565  # log2(|a3|)

# per-group engine assignment (ring-sim optimized for measured cadences)
PATTERN = "M" * 32
ACT_COLS = {"A": 2048, "M": 1536, "D": 0}

_cached = {}


def _register_ops():
    import concourse.dve_ops as dve_ops

    have = {op.name for op in dve_ops.OPS}
    if "ANT_EXPC1" in have:
        p1 = next(o for o in dve_ops.OPS if o.name == "ANT_EXPC1")
        p2 = next(o for o in dve_ops.OPS if o.name == "ANT_EXP2H")
        return p1, p2

    from concourse.dve_spec import (
        AluOp,
        Bin,
        C0,
        C1,
        C2,
        One,
        Spec,
        Src0,
        lower,
        relu,
        _has_src1,
    )
    from concourse.dve_uop import DveOpSpec

    def p1_ref(in0, in1, s0, s1, imm2):
        return np.maximum(
            in0.astype(np.float32) + np.asarray(s0, np.float32), 0.0
        )

    p1_spec = Spec(body=relu(Src0 + C0), reference=p1_ref)

    t_ = Bin(AluOp.BITWISE_OR, Src0, One)
    u_ = C0 - t_
    v_ = u_ * t_
    w_ = v_ + C1
    x_ = w_ * t_
    y_ = x_ + C2
    r_ = y_ * Src0

    def p2_ref(in0, in1, s0, s1, imm2):
        i = np.ascontiguousarray(in0).view(np.int32)
        one = np.float32(1.0).view(np.int32)
        t = (i | one).view(np.float32).astype(np.float32)
        e2 = np.ascontiguousarray(in0).view(np.float32).astype(np.float32)
        body = (
            (np.asarray(s0, np.float32) - t) * t + np.asarray(s1, np.float32)
        ) * t + np.float32(imm2)
        body = body * e2
        return body, body.reshape(body.shape[0], -1).sum(axis=-1, keepdims=True)

    p2_spec = Spec(body=r_, accum=AluOp.ADD, reference=p2_ref)

    out = []
    for name, spec in [("ANT_EXPC1", p1_spec), ("ANT_EXP2H", p2_spec)]:
        opcode = max(dve_ops._SUB_OPCODE_FOR_NAME.values()) + 1
        shas = {}
        for ver in ("v3", "v4"):
            s = DveOpSpec(
                name=name,
                opcode=opcode,
                uops=lower(spec, ver=ver),
                rd1_en=_has_src1(spec),
            )
            shas[ver] = s.sha(ver)
        op = dve_ops.DveOp(name, spec, subdim=False, uops_sha=shas)
        dve_ops.OPS.append(op)
        dve_ops._SUB_OPCODE_FOR_NAME[name] = opcode
        dve_ops.CUSTOM_DVE_SPECS[name] = spec
        out.append(op)
    return tuple(out)


def _build_program():
    import concourse.bass as bass
    import concourse.mybir as mybir
    from contextlib import ExitStack

    p1op, p2op = _register_ops()

    nc = bass.Bass()
    f32 = mybir.dt.float32
    i32 = mybir.dt.int32
    bf16 = mybir.dt.bfloat16

    LN2_SC = float(np.log(2.0) / (1 << 23))

    # groups in h-outer order (g = h*NT + t): second m-half deferred
    groups = []
    for h in range(M // GW):
        for t in range(NT):
            g = h * NT + t
            groups.append((t, h, ACT_COLS[PATTERN[g]]))
    # per-group instr-count prefixes (for count-indexed semaphores)
    a_cnt = []  # ACT instrs through group g inclusive
    d_cnt = []  # DVE p1 instrs through group g inclusive
    ca = cd = 0
    for _, _, acols in groups:
        if acols > 0:
            ca += 1
        if acols < GW:
            cd += 1
        a_cnt.append(ca)
        d_cnt.append(cd)

    la = nc.declare_dram_parameter("la", [K, NSHARD], bf16, isOutput=False)
    R = nc.declare_dram_parameter("R", [K, M], bf16, isOutput=False)
    # xqn: 3 scalar-broadcast cols + permuted queries [128, NT*D]:
    #   [0] : -1/bw                      (ACT bias mult)
    #   [1] : -2^23*log2e/bw             (C0 bias mult)
    #   [2] : (127+LOG2_NEG_C3)*2^23     (C0 bias add)
    XQN_W = 3 + NT * D
    xqn = nc.declare_dram_parameter("xqn", [128, XQN_W], f32, isOutput=False)
    res = nc.declare_dram_parameter("res", [128, NT], f32, isOutput=True)

    LW_ = NSHARD // 2  # la DMA chunk cols
    XC = NT * D // 4  # xqn query cols per slice (128)
    # R chunks: 4x512 (first m-half, fine-grained for the head) + 2x1024
    R_CH = [(0, 512), (512, 1024), (1024, 1536), (1536, 2048),
            (2048, 3072), (3072, 4096)]

    with ExitStack() as ctx:
        la_sb = ctx.enter_context(nc.sbuf_tensor([K, NSHARD], bf16))
        R_sb = ctx.enter_context(nc.sbuf_tensor([K, M], bf16))
        xqn_sb = ctx.enter_context(nc.sbuf_tensor([128, XQN_W], f32))
        sq_sb = ctx.enter_context(nc.sbuf_tensor([128, NT * D], f32))
        nx2r = ctx.enter_context(nc.sbuf_tensor([128, NT], f32))
        bias_act = ctx.enter_context(nc.sbuf_tensor([128, NT], f32))
        bias_c0 = ctx.enter_context(nc.sbuf_tensor([128, NT], f32))
        accA = ctx.enter_context(nc.sbuf_tensor([128, NG], f32))
        accD = ctx.enter_context(nc.sbuf_tensor([128, NG], f32))
        accS = ctx.enter_context(nc.sbuf_tensor([128, NG], f32))
        res_sb = ctx.enter_context(nc.sbuf_tensor([128, NT], f32))
        warmT = ctx.enter_context(nc.sbuf_tensor([1, 1], f32))
        escr0 = ctx.enter_context(nc.sbuf_tensor([128, GW], bf16))
        escr1 = ctx.enter_context(nc.sbuf_tensor([128, GW], bf16))
        escr = [escr0, escr1]
        iscr = [
            ctx.enter_context(nc.sbuf_tensor(f"iscr{_i}", [128, GW], i32))
            for _i in range(4)
        ]
        dscr = [
            ctx.enter_context(nc.sbuf_tensor(f"dscr{_i}", [128, GW], bf16))
            for _i in range(4)
        ]
        ps0 = ctx.enter_context(nc.psum_tensor("ps0", [128, GW], f32))
        ps1 = ctx.enter_context(nc.psum_tensor("ps1", [128, GW], f32))
        ps = [ps0, ps1]

        s_xq = [ctx.enter_context(nc.semaphore(f"s_xq{i}")) for i in range(4)]
        s_la = [ctx.enter_context(nc.semaphore(f"s_la{i}")) for i in range(3)]
        s_R = [ctx.enter_context(nc.semaphore(f"s_R{i}")) for i in range(6)]
        sem_out = ctx.enter_context(nc.semaphore("sem_out"))
        s_warm = ctx.enter_context(nc.semaphore("s_warm"))
        s_dve = ctx.enter_context(nc.semaphore("s_dve"))
        s_bias = ctx.enter_context(nc.semaphore("s_bias"))
        s_pa = ctx.enter_context(nc.semaphore("s_pa"))  # ACT share ready
        s_pd = ctx.enter_context(nc.semaphore("s_pd"))  # DVE share ready
        s_act = ctx.enter_context(nc.semaphore("s_act"))  # ACT instrs done
        s_dv1 = ctx.enter_context(nc.semaphore("s_dv1"))  # p1 done count
        s_dv2 = ctx.enter_context(nc.semaphore("s_dv2"))  # p2 done count
        s_fin = ctx.enter_context(nc.semaphore("s_fin"))
        block = ctx.enter_context(nc.Block())

        @block.sync
        def _(sync):
            # xqn slice 0 heads the critical path to the first activation
            # (bias chain); R chunks 1-2 are needed by group 0's ACT share
            sync.dma_start(
                xqn_sb[:, 0 : 3 + XC], xqn[:, 0 : 3 + XC]
            ).then_inc(s_xq[0], 16)
            for i in (1, 2, 3):
                a, b = R_CH[i]
                sync.dma_start(R_sb[:, a:b], R[:, a:b]).then_inc(s_R[i], 16)
            for i in range(1, 4):
                sync.dma_start(
                    xqn_sb[:, 3 + i * XC : 3 + (i + 1) * XC],
                    xqn[:, 3 + i * XC : 3 + (i + 1) * XC],
                ).then_inc(s_xq[i], 16)
            sync.dma_start(
                la_sb[:, LW_ : 2 * LW_], la[:, LW_ : 2 * LW_]
            ).then_inc(s_la[1], 16)
            # second m-half deferred: not consumed until ~halfway
            sync.wait_ge(s_la[1], 16)
            for i in (4, 5):
                a, b = R_CH[i]
                sync.dma_start(R_sb[:, a:b], R[:, a:b]).then_inc(s_R[i], 16)
            sync.wait_ge(s_fin, 1)
            sync.dma_start(res[:], res_sb[:]).then_inc(sem_out, 16)
            sync.wait_ge(sem_out, 16)

        @block.vector
        def _(vector):
            # head: scratch + per-slice nx2/bias chain
            nc.vector.memset(escr0[:, 0:MCHUNK], 0.0).then_inc(s_dve, 1)
            nc.vector.memset(accD[:], 0.0)
            nc.vector.memset(accA[:], 0.0)
            nsem = 1
            for sl in range(4):
                c0, c1 = 3 + sl * XC, 3 + (sl + 1) * XC
                t0, t1 = sl * 4, (sl + 1) * 4
                vector.wait_ge(s_xq[sl], 16)
                if sl == 0:
                    pass  # scalar cols arrive with slice 0
                nc.vector.tensor_tensor(
                    sq_sb[:, sl * 4 * D : (sl + 1) * 4 * D],
                    xqn_sb[:, c0:c1],
                    xqn_sb[:, c0:c1],
                    op=mybir.AluOpType.mult,
                ).then_inc(s_dve, 1)
                nsem += 1
                vector.wait_ge(s_dve, nsem)
                nc.vector.tensor_reduce(
                    nx2r[:, t0:t1],
                    sq_sb[:, sl * 4 * D : (sl + 1) * 4 * D].rearrange(
                        "p (t d) -> p t d", d=D
                    ),
                    axis=mybir.AxisListType.X,
                    op=mybir.AluOpType.add,
                ).then_inc(s_dve, 1)
                nsem += 1
                vector.wait_ge(s_dve, nsem)
                nc.vector.tensor_scalar(
                    bias_act[:, t0:t1],
                    nx2r[:, t0:t1],
                    xqn_sb[:, 0:1],
                    None,
                    op0=mybir.AluOpType.mult,
                ).then_inc(s_dve, 1)
                nsem += 1
                nc.vector.tensor_scalar(
                    bias_c0[:, t0:t1],
                    nx2r[:, t0:t1],
                    xqn_sb[:, 1:2],
                    xqn_sb[:, 2:3],
                    op0=mybir.AluOpType.mult,
                    op1=mybir.AluOpType.add,
                ).then_inc(s_bias, 1)
            # main loop: DVE shares; pass2 one group behind pass1
            dlist = [
                (g, t, acols)
                for g, (t, h, acols) in enumerate(groups)
                if acols < GW
            ]
            prev = None
            seen_sl = -1
            for k, (g, t, acols) in enumerate(dlist):
                dcols = GW - acols
                pg = ps[g % 2]
                if t // 4 > seen_sl and g < NT:
                    seen_sl = t // 4
                    vector.wait_ge(s_bias, seen_sl + 1)
                vector.wait_ge(s_pd, k + 1)
                if k >= 4:
                    vector.wait_ge(s_dv2, k - 3)
                nc.vector._custom_dve(
                    p1op,
                    out=iscr[k % 4][:, 0:dcols],
                    in0=pg[:, acols:GW],
                    s0=bias_c0[:, t : t + 1],
                ).then_inc(s_dv1, 1)
                if prev is not None:
                    kp, gp, dcp = prev
                    vector.wait_ge(s_dv1, kp + 1)
                    nc.vector._custom_dve(
                        p2op,
                        out=dscr[kp % 4][:, 0:dcp],
                        in0=iscr[kp % 4][:, 0:dcp].bitcast(f32),
                        s0=A2_C,
                        s1=A1_C,
                        imm2=A0_C,
                        accum_out=accD[:, gp : gp + 1],
                    ).then_inc(s_dv2, 1)
                prev = (k, g, dcols)
            kp, gp, dcp = prev
            vector.wait_ge(s_dv1, kp + 1)
            nc.vector._custom_dve(
                p2op,
                out=dscr[kp % 4][:, 0:dcp],
                in0=iscr[kp % 4][:, 0:dcp].bitcast(f32),
                s0=A2_C,
                s1=A1_C,
                imm2=A0_C,
                accum_out=accD[:, gp : gp + 1],
            ).then_inc(s_dv2, 1)
            # tail: fold accA + accD over m-halves
            vector.wait_ge(s_act, a_cnt[-1])
            vector.wait_ge(s_dv2, len(dlist))
            nc.vector.tensor_tensor(
                accS[:], accA[:], accD[:], op=mybir.AluOpType.add
            ).then_inc(s_dve, 1)
            vector.wait_ge(s_dve, nsem + 1)
            nc.vector.tensor_tensor(
                res_sb[:],
                accS[:, 0:NT],
                accS[:, NT : 2 * NT],
                op=mybir.AluOpType.add,
            ).then_inc(s_fin, 1)

        @block.scalar
        def _(scalar):
            # exp table warm first: ACT_TABLE_LOAD (~2.7us) runs on the
            # engine while the sequencer issues the R0/la0 DMAs below
            nc.scalar.memzero(warmT[:]).then_inc(s_warm, 1)
            scalar.wait_ge(s_warm, 1)
            nc.scalar.activation(
                warmT[:], warmT[:], mybir.ActivationFunctionType.Exp
            )
            a, b = R_CH[0]
            scalar.dma_start(R_sb[:, a:b], R[:, a:b]).then_inc(s_R[0], 16)
            scalar.dma_start(la_sb[:, 0:LW_], la[:, 0:LW_]).then_inc(
                s_la[0], 16
            )
            ai = 0
            seen_sl = -1
            for g, (t, h, acols) in enumerate(groups):
                if acols == 0:
                    continue
                pg = ps[g % 2]
                if t // 4 > seen_sl and g < NT:
                    seen_sl = t // 4
                    scalar.wait_ge(s_bias, seen_sl + 1)
                scalar.wait_ge(s_pa, ai + 1)
                nc.scalar.activation(
                    escr[g % 2][:, 0:acols],
                    pg[:, 0:acols],
                    mybir.ActivationFunctionType.Exp,
                    bias=bias_act[:, t : t + 1],
                    scale=LN2_SC,
                    accum_out=accA[:, g : g + 1],
                ).then_inc(s_act, 1)
                ai += 1

        @block.tensor
        def _(tensor):
            # warm the PE clock with dummy matmuls on the memset scratch
            tensor.wait_ge(s_dve, 1)
            for _w in range(6):
                nc.tensor.matmul(
                    ps0[:, 0:128],
                    escr0[:, 0:128],
                    escr0[:, 0:128],
                    start=True,
                    stop=True,
                )
            for g, (t, h, acols) in enumerate(groups):
                if g < NT and t % 8 == 0:
                    tensor.wait_ge(s_la[t // 8], 16)
                pg = ps[g % 2]
                lsl = slice(t * 128, (t + 1) * 128)
                na = acols // MCHUNK
                if g >= 2:
                    if a_cnt[g - 2] > 0:
                        tensor.wait_ge(s_act, a_cnt[g - 2])
                    if d_cnt[g - 2] > 0:
                        tensor.wait_ge(s_dv1, d_cnt[g - 2])
                for j in range(4):
                    if t == 0:
                        if h == 0:
                            tensor.wait_ge(s_R[j], 16)
                        elif j % 2 == 0:
                            tensor.wait_ge(s_R[4 + j // 2], 16)
                    m0 = h * GW + j * MCHUNK
                    mm = nc.tensor.matmul(
                        pg[:, j * MCHUNK : (j + 1) * MCHUNK],
                        la_sb[:, lsl],
                        R_sb[:, m0 : m0 + MCHUNK],
                        start=True,
                        stop=True,
                    )
                    if na > 0 and j == na - 1:
                        mm.then_inc(s_pa, 1)
                    if na < 4 and j == 3:
                        mm.then_inc(s_pd, 1)

    return nc


def _bf16_split2(x):
    import ml_dtypes

    bf = ml_dtypes.bfloat16
    x = x.astype(np.float32)
    p1 = x.astype(bf)
    rem = x - p1.astype(np.float32)
    p2 = rem.astype(bf)
    return p1, p2


def _bf16_split3(x):
    import ml_dtypes

    bf = ml_dtypes.bfloat16
    x = x.astype(np.float32)
    p1 = x.astype(bf)
    rem = x - p1.astype(np.float32)
    p2 = rem.astype(bf)
    rem2 = rem - p2.astype(np.float32)
    p3 = rem2.astype(bf)
    return p1, p2, p3


def _bandwidth_np(X_fit):
    b, n, d = X_fit.shape
    flat = np.asarray(X_fit, dtype=np.float64).reshape(-1)
    q = np.quantile(flat, 0.75) - np.quantile(flat, 0.25)
    std = np.std(
        np.asarray(X_fit, dtype=np.float64).reshape(b, -1), axis=1, ddof=1
    )
    return (0.9 * np.minimum(std, q / 1.34) / (n**0.2)).astype(np.float32)


def _host_prep(X_query, X_fit):
    import ml_dtypes

    bf = ml_dtypes.bfloat16
    X_query = np.asarray(X_query, dtype=np.float32)
    X_fit = np.asarray(X_fit, dtype=np.float32)
    bw = _bandwidth_np(X_fit)  # [B]
    log2e = np.float64(np.log2(np.e))

    in_maps = []
    for c in range(NCORES):
        b = c // SHARDS_PER_BATCH
        s = c % SHARDS_PER_BATCH
        XQ = X_query[b, s * NSHARD : (s + 1) * NSHARD]  # [2048, 32]
        XF = X_fit[b]  # [4096, 32]
        Sf = np.float64(2.0) * (1 << 23) * log2e / np.float64(bw[b])

        # permuted queries: tile t / partition p <-> query row p*NT + t
        XQp = XQ.reshape(128, NT, D).transpose(1, 0, 2).reshape(NSHARD, D)
        Q = np.ascontiguousarray(XQp.T.astype(np.float32))  # [32, 2048]
        q1, q2 = _bf16_split2(Q)
        F = np.ascontiguousarray(
            (XF.T.astype(np.float64) * Sf).astype(np.float32)
        )  # [32, 4096]
        F1, F2 = _bf16_split2(F)
        # nmu2 in f32 to match the reference's f32 sum
        nmu2 = (XF.astype(np.float32) ** 2).sum(axis=1).astype(np.float32)
        nm_full = (nmu2.astype(np.float64) * (Sf / 2.0)).astype(np.float32)
        nm1, nm2, nm3 = _bf16_split3(nm_full[None, :])

        la_np = np.empty((K, NSHARD), dtype=bf)
        la_np[0:32] = q1
        la_np[32:64] = q1
        la_np[64:96] = q2
        la_np[96:99] = np.float32(-1.0)
        R_np = np.empty((K, M), dtype=bf)
        R_np[0:32] = F1
        R_np[32:64] = F2
        R_np[64:96] = F1
        R_np[96] = nm1
        R_np[97] = nm2
        R_np[98] = nm3

        inv_bw = np.float64(1.0) / np.float64(bw[b])
        xqn = np.empty((128, 3 + NT * D), dtype=np.float32)
        xqn[:, 0] = np.float32(-inv_bw)
        xqn[:, 1] = np.float32(-(1 << 23) * log2e * inv_bw)
        xqn[:, 2] = np.float32((127.0 + LOG2_NEG_C3) * (1 << 23))
        xqn[:, 3:] = XQ.reshape(128, NT * D)

        in_maps.append({"la": la_np, "R": R_np, "xqn": xqn})
    return in_maps


def _gather(results):
    out = np.empty((B, N), dtype=np.float32)
    for c in range(NCORES):
        b = c // SHARDS_PER_BATCH
        s = c % SHARDS_PER_BATCH
        res = np.asarray(results[c]["res"], dtype=np.float32)  # [128, NT]
        out[b, s * NSHARD : (s + 1) * NSHARD] = res.reshape(NSHARD)
    return out


def kernel(X_query, X_fit):
    from concourse.bass_utils import run_bass_kernel_spmd
    from concourse.library_overlay import lower_extended_insts

    if "nc" not in _cached:
        nc = _build_program()
        lower_extended_insts(nc)
        _cached["nc"] = nc
    nc = _cached["nc"]
    in_maps = _host_prep(X_query, X_fit)
    out = run_bass_kernel_spmd(nc, in_maps, list(range(NCORES)))
    result = _gather(out.results)
    if not np.isfinite(result).all():
        # defensive re-run (transient device-state residue)
        out = run_bass_kernel_spmd(nc, in_maps, list(range(NCORES)))
        result = _gather(out.results)
    return result
